# revision 12
# baseline (speedup 1.0000x reference)
"""Trainium2 Bass kernel for nn_Melody_RNN (B=64, S=512, A=20, V=130, E=H=64, L=2).

Structure exploited (all implied by the reference's exact semantics):
  * Only embedding rows for inputs[:,0] / inputs[:,1] are used; the LSTM runs
    exactly 2 timesteps (zero initial state, so the forget gate is dead).
  * The torch cat+view memory reinterpretations make h_steps/c_steps rows a
    small periodic table (period 64 in s, independent of b), with batch-0
    special rows for s<64.
  * The attention-mask bug makes softmax exactly uniform, so
    attn[b,s] = (1/A) * sum_{t=max(0,s-20)}^{s-1} Wh[b,t] + Wc[b,s].
  * outs[b,s] therefore equals generic rows OG[0:84] + 64-periodic repetition,
    with 84 special rows for batch 0 only.

Kernel v6 notes (changes vs v4 baseline):
  * TWO fused input DMAs (sync: xs+wih0 so the LSTM can start early; scalar:
    everything else). LSTM gate biases and table biases (Whb/A, Wcb) folded
    into the matmuls via a ones row (row 64 of xs / hcat / ccat).
  * Output staged in bf16 with even/odd row interleave: og2[p, 0:130] = row 2p,
    og2[p, 130:260] = row 2p+1, so each DMA descriptor moves 2 consecutive
    output rows = 520 B (stays at HBM line-rate) at half the f32 bytes.
  * 9 output DMAs (6 periodic + tail + heads + blended head0), all HWDGE
    (sync/scalar), periodic sources alternating between two partition bases
    to engage all 16 SDMA engines; host unshards and upcasts bf16 -> f32.
  * Engine queues ordered so the generic path (tables -> window tree -> decode
    -> og2 staging -> DMAs) is never stuck behind the batch-0 blend path.

SPMD: 8 cores, identical program; per-core input differs only in the mvec
column (1.0 on core 0 -> blends the batch-0 special block).
"""

import sys
import numpy as np

if "/root/.axon_site/_ro/trn_rl_repo" not in sys.path:
    sys.path.insert(0, "/root/.axon_site/_ro/trn_rl_repo")

B, S, A = 64, 512, 20
V, E, H = 130, 64, 64
NCORES = 8
BPC = B // NCORES  # batches per core

# input 1 [65, _W1]: LSTM layer-0 operands (row 64 = ones / biases)
_XS = 0        # [65, 128] = [x0T | x1T], row 64 = 1.0
_WIH0 = 128    # [65, 192] gates (i,g,o), row 64 = bih0+bhh0 packed
_W1 = 320
# input 2 [65, _W2]: everything else
_WIH1 = 0      # [65, 192] gates (i,g,o), row 64 = bih1+bhh1 packed
_WHW = 192     # [65, 128] = [Whw[:, :64].T | Whw[:, 64:].T] / A, row 64 = [Whb/A | 0]
_WCW = 320     # [65, 128] = Wcw halves transposed, row 64 = [Wcb | 0]
_DECA = 448    # [65, 130] = decw[:, 0:64].T (attn-half rows), row 64 = 0
_DECB = 578    # [65, 130] = decw[:, 64:128].T, row 64 = decb
_MVEC = 708    # [65, 1] blend selector column (1.0 on core 0)
_W2 = 709

_NC_CACHE = {}


def _build_nc():
    import concourse.bass as bass
    import concourse.bacc as bacc
    import concourse.mybir as mybir
    from concourse.tile import TileContext

    f32 = mybir.dt.float32
    bf16 = mybir.dt.bfloat16
    AF = mybir.ActivationFunctionType

    nc = bacc.Bacc("TRN2", target_bir_lowering=False, debug=False)

    d_in1 = nc.dram_tensor("inp1", [65, _W1], f32, kind="ExternalInput")
    d_in2 = nc.dram_tensor("inp2", [65, _W2], f32, kind="ExternalInput")
    d_out = nc.dram_tensor("out", [BPC * S, V], bf16, kind="ExternalOutput")

    PAD = 20 + 103  # 20 zero cols + WhSeq t=0..102
    SLOT = S * V    # elements per output slot (66560)

    with TileContext(nc) as tc:
        with (
            tc.tile_pool(name="sbuf", bufs=1) as pool,
            tc.tile_pool(name="psum", bufs=1, space="PSUM") as pp,
        ):
            # ---- input DMAs: layer-0 operands first so the LSTM can start --
            inp1 = pool.tile([65, _W1], f32)
            inp2 = pool.tile([65, _W2], f32)
            nc.sync.dma_start(out=inp1[:], in_=d_in1[:])
            nc.scalar.dma_start(out=inp2[:], in_=d_in2[:])

            # preload Sigmoid/Tanh ACT tables while the input DMAs are in flight
            ones = pool.tile([1, 2], f32)
            dummy = pool.tile([1, 2], f32)
            nc.vector.memset(ones[:], 1.0)
            nc.scalar.activation(dummy[0:1, 0:1], ones[0:1, 0:1], AF.Sigmoid)
            nc.scalar.activation(dummy[0:1, 1:2], ones[0:1, 0:1], AF.Tanh)

            # ---- persistent tiles + early memsets (off critical path) ----
            hcat = pool.tile([65, 256], f32)
            ccat = pool.tile([65, 256], f32)
            padG = pool.tile([64, PAD], f32)
            pad0 = pool.tile([64, PAD], f32)
            outG = pool.tile([65, 84], f32)
            outB = pool.tile([65, 84], f32)
            nc.vector.memset(hcat[64:65, :], 1.0)
            nc.vector.memset(ccat[64:65, :], 1.0)
            nc.gpsimd.memset(padG[:, 0:20], 0.0)
            nc.gpsimd.memset(pad0[:, 0:20], 0.0)
            nc.vector.memset(outG[64:65, :], 1.0)
            nc.vector.memset(outB[64:65, :], 1.0)

            # ---- LSTM: both timesteps batched; biases via ones-row matmul ---
            # hcat/ccat cols [l0s0|l0s1|l1s0|l1s1], row 64 = 1.0
            def lstm_layer(rhsT, wp, tag, dst_off):
                ps0 = pp.tile([128, 128], f32, tag="gates")   # [i|g]
                ps1 = pp.tile([64, 128], f32, tag="gateso")   # [o]
                nc.tensor.matmul(ps0[:], wp[:, 0:128], rhsT, start=True, stop=True)
                nc.tensor.matmul(ps1[:], wp[:, 128:192], rhsT, start=True, stop=True)
                sig_i = pool.tile([64, 128], f32, tag=f"sigi{tag}")
                tanh_g = pool.tile([64, 128], f32, tag=f"tanhg{tag}")
                sig_o = pool.tile([64, 128], f32, tag=f"sigo{tag}")
                tanh_c = pool.tile([64, 128], f32, tag=f"tanhc{tag}")
                cc = ccat[0:64, dst_off:dst_off + 128]
                hh = hcat[0:64, dst_off:dst_off + 128]
                nc.scalar.activation(tanh_g[:], ps0[64:128, :], AF.Tanh)
                nc.scalar.activation(sig_i[:], ps0[0:64, :], AF.Sigmoid)
                nc.vector.tensor_mul(cc, sig_i[:], tanh_g[:])
                nc.scalar.activation(sig_o[:], ps1[:], AF.Sigmoid)
                nc.scalar.activation(tanh_c[:], cc, AF.Tanh)
                nc.vector.tensor_mul(hh, sig_o[:], tanh_c[:])

            lstm_layer(inp1[:, _XS:_XS + 128], inp1[:, _WIH0:_WIH0 + 192], "l0", 0)
            lstm_layer(hcat[:, 0:128], inp2[:, _WIH1:_WIH1 + 192], "l1", 128)
            # hcat cols: h0l0 0:64, h1l0 64:128, h0l1 128:192, h1l1 192:256
            out0T = hcat[0:64, 128:192]
            out1T = hcat[0:64, 192:256]

            # ---- outputs-half row tiles [65, 84] (row 64 = ones for decb) --
            outZ = pool.tile([64, 84], f32)
            nc.gpsimd.tensor_copy(outG[0:64, 0:64], out1T)
            nc.gpsimd.tensor_copy(outG[0:64, 64:84], out1T[:, 0:20])
            nc.gpsimd.tensor_copy(outZ[:, 0:64], out0T)
            nc.gpsimd.tensor_copy(outZ[:, 64:84], out1T[:, 0:20])

            # ---- decoder psums (even/odd s split); outputs-half first ----
            DECB = inp2[:, _DECB:_DECB + V]          # [65, 130]
            DECA = inp2[0:64, _DECA:_DECA + V]       # [64, 130]
            ogPe = pp.tile([42, V], f32, tag="ogpe")
            ogPo = pp.tile([42, V], f32, tag="ogpo")
            obPe = pp.tile([42, V], f32, tag="obpe")
            obPo = pp.tile([42, V], f32, tag="obpo")
            # ---- all 8 row tables in 2 psums (bias via ones row) ----
            def tables(cat, wcol, tag):
                p = pp.tile([64, 128], f32, tag=tag)
                w = inp2[:, wcol:wcol + 128]
                nc.tensor.matmul(p[:], w[:, 0:64], cat[:, 0:256:2], start=True, stop=False)
                nc.tensor.matmul(p[:], w[:, 64:128], cat[:, 1:256:2], start=False, stop=True)
                return p

            whT = tables(hcat, _WHW, "tabh")
            wcT = tables(ccat, _WCW, "tabc")
            # outputs-half decode AFTER the tables so PE never interleaves
            # these into the critical tabh accumulation pair
            nc.tensor.matmul(ogPe[:], outG[:, 0:84:2], DECB, start=True, stop=False)
            nc.tensor.matmul(ogPo[:], outG[:, 1:84:2], DECB, start=True, stop=False)
            whTt = whT[:].tensor
            wcTt = wcT[:].tensor

            # ---- padded Wh sequences + Wc rows (plain strided copies) ----
            # psum col blocks: [whs0 | whp0 | whs1 | whp1]
            wcG = pool.tile([64, 84], f32)
            wc0 = pool.tile([64, 84], f32)

            def two_block(t, off):
                # [t[:, off:off+32] | t[:, off+64:off+96]] as one 3-dim AP
                return bass.AP(t, off, [[128, 64], [64, 2], [1, 32]])

            nc.vector.tensor_copy(padG[:, 20:84], two_block(whTt, 32))
            nc.vector.tensor_copy(padG[:, 84:103], bass.AP(whTt, 32, [[128, 64], [1, 19]]))
            nc.scalar.copy(pad0[:, 20:84], two_block(whTt, 0))
            nc.scalar.copy(pad0[:, 84:103], bass.AP(whTt, 32, [[128, 64], [1, 19]]))
            nc.scalar.copy(wcG[:, 0:64], two_block(wcTt, 32))
            nc.scalar.copy(wcG[:, 64:84], bass.AP(wcTt, 32, [[128, 64], [1, 20]]))
            nc.scalar.copy(wc0[:, 0:64], two_block(wcTt, 0))
            nc.scalar.copy(wc0[:, 64:84], bass.AP(wcTt, 32, [[128, 64], [1, 20]]))

            # ---- sliding 20-window sums via shift-add tree ----
            def window20(pad, eng, tag):
                t1 = pool.tile([64, 102], f32, tag=f"t1{tag}")
                t2 = pool.tile([64, 100], f32, tag=f"t2{tag}")
                t4 = pool.tile([64, 96], f32, tag=f"t4{tag}")
                t8 = pool.tile([64, 88], f32, tag=f"t8{tag}")
                w20 = pool.tile([64, 84], f32, tag=f"w20{tag}")
                eng.tensor_add(t1[:], pad[:, 0:102], pad[:, 1:103])
                eng.tensor_add(t2[:], t1[:, 0:100], t1[:, 2:102])
                eng.tensor_add(t4[:], t2[:, 0:96], t2[:, 4:100])
                eng.tensor_add(t8[:], t4[:, 0:88], t4[:, 8:96])
                eng.tensor_add(w20[:], t8[:, 0:84], t2[:, 16:100])
                return w20

            w20G = window20(padG, nc.vector, "g")
            w20_0 = window20(pad0, nc.gpsimd, "z")

            # ---- attn halves [64, 84] ----
            attnG = pool.tile([64, 84], f32)
            attnZ = pool.tile([64, 84], f32)
            nc.vector.tensor_add(attnG[:], w20G[:], wcG[:])
            nc.gpsimd.tensor_add(attnZ[:], w20_0[:], wc0[:])

            # ---- finish generic decode ----
            nc.tensor.matmul(ogPe[:], attnG[:, 0:84:2], DECA, start=False, stop=True)
            nc.tensor.matmul(ogPo[:], attnG[:, 1:84:2], DECA, start=False, stop=True)

            # ---- bf16 staging: og2[p, 0:130] = row 2p, [130:260] = row 2p+1.
            #      Partitions 64:106 = aligned compute replica so half the
            #      periodic DMAs read odd-numbered SDMA engines' partitions
            #      (engine k serves fixed SBUF partitions; 64:128 -> odd). ----
            og2 = pool.tile([106, 260], bf16)
            ob2 = pool.tile([42, 260], bf16)
            nc.scalar.copy(og2[0:42, 0:130], ogPe[:])
            nc.scalar.copy(og2[0:42, 130:260], ogPo[:])
            nc.scalar.copy(og2[64:106, :], og2[0:42, :])
            og2t = og2[:].tensor
            ob2t = ob2[:].tensor

            # ---- output DMAs (bf16, 520B descriptors = 2 rows each),
            #      all on HWDGE queues (SWDGE drains far slower) ----
            def dst(row0, nparts, slot0, nslots):
                return bass.AP(d_out, slot0 * SLOT + row0 * V,
                               [[260, nparts], [SLOT, nslots], [1, 260]])

            def src(t, part0, nparts, nslots):
                return bass.AP(t, part0 * 260, [[260, nparts], [0, nslots], [1, 260]])

            # periodic rows 84+64k <- OG[20:84] (k = 0..5), all 8 slots each;
            # even k from the base copy (parts 10:42), odd k from the replica
            # (parts 74:106) to spread across all 16 SDMA engines
            nc.sync.dma_start(out=dst(84, 32, 0, 8), in_=src(og2t, 10, 32, 8))
            nc.scalar.dma_start(out=dst(148, 32, 0, 8), in_=src(og2t, 74, 32, 8))
            nc.sync.dma_start(out=dst(212, 32, 0, 8), in_=src(og2t, 10, 32, 8))
            nc.scalar.dma_start(out=dst(276, 32, 0, 8), in_=src(og2t, 74, 32, 8))
            nc.sync.dma_start(out=dst(340, 32, 0, 8), in_=src(og2t, 10, 32, 8))
            nc.scalar.dma_start(out=dst(404, 32, 0, 8), in_=src(og2t, 74, 32, 8))
            # tail rows 468:512 <- OG[20:64] (from the replica: odd engines)
            nc.sync.dma_start(out=dst(468, 22, 0, 8), in_=src(og2t, 74, 22, 8))
            # heads: slots 1..7 generic
            nc.sync.dma_start(out=dst(0, 42, 1, 7), in_=src(og2t, 0, 42, 7))

            # ---- blend batch-0 variants: X_B = X_G + mvec*(X_0 - X_G) ----
            MV = inp2[0:64, _MVEC:_MVEC + 1]
            attnB = pool.tile([64, 84], f32)
            dA = pool.tile([64, 84], f32)
            dO = pool.tile([64, 84], f32)
            nc.vector.tensor_sub(dO[:], outZ[:], outG[0:64, :])
            nc.vector.tensor_scalar_mul(dO[:], dO[:], MV)
            nc.vector.tensor_add(outB[0:64, :], outG[0:64, :], dO[:])
            nc.vector.tensor_sub(dA[:], attnZ[:], attnG[:])
            nc.vector.tensor_scalar_mul(dA[:], dA[:], MV)
            nc.vector.tensor_add(attnB[:], attnG[:], dA[:])

            nc.tensor.matmul(obPe[:], outB[:, 0:84:2], DECB, start=True, stop=False)
            nc.tensor.matmul(obPo[:], outB[:, 1:84:2], DECB, start=True, stop=False)
            nc.tensor.matmul(obPe[:], attnB[:, 0:84:2], DECA, start=False, stop=True)
            nc.tensor.matmul(obPo[:], attnB[:, 1:84:2], DECA, start=False, stop=True)
            nc.vector.tensor_copy(ob2[:, 0:130], obPe[:])
            nc.vector.tensor_copy(ob2[:, 130:260], obPo[:])

            # head slot 0 blended
            nc.scalar.dma_start(
                out=bass.AP(d_out, 0, [[260, 42], [1, 260]]),
                in_=bass.AP(ob2t, 0, [[260, 42], [1, 260]]))

    nc.compile()
    return nc


def _get_nc():
    if "nc" not in _NC_CACHE:
        _NC_CACHE["nc"] = _build_nc()
    return _NC_CACHE["nc"]


def _host_reference_fallback(inputs):
    """Pure-numpy replica of the reference for steps != 512 (never hit with the
    canonical setup_inputs, which fixes lengths = 512)."""
    emb = inputs["emb"]; L = 2
    Ls = np.asarray(inputs["lengths"]); steps = int(Ls.max()); batch = inputs["inputs"].shape[0]
    layers = [(inputs["Wih0"], inputs["bih0"], inputs["bhh0"]),
              (inputs["Wih1"], inputs["bih1"], inputs["bhh1"])]
    sig = lambda z: 1.0 / (1.0 + np.exp(-z))

    def step(x):
        hs, cs = [], []
        inp = x
        for (Wih, bih, bhh) in layers:
            g = inp @ Wih.T + bih + bhh
            i, f, gg, o = np.split(g, 4, axis=-1)
            c = sig(i) * np.tanh(gg)
            h = sig(o) * np.tanh(c)
            hs.append(h); cs.append(c); inp = h
        return inp.astype(np.float32), np.stack(hs).astype(np.float32), np.stack(cs).astype(np.float32)

    x0 = emb[inputs["inputs"][:, 0]]
    x1 = emb[inputs["inputs"][:, 1]]
    out0, h0, c0 = step(x0)
    out1, h1, c1 = step(x1)
    outputs = np.concatenate(
        [out0[None], np.broadcast_to(out1[None], (steps - 1, batch, H))], 0
    ).reshape(batch, steps, H)
    h_steps = np.concatenate(
        [h0, np.broadcast_to(h1[None], (steps - 1, L, batch, H)).reshape((steps - 1) * L, batch, H)], 0
    ).reshape(batch, steps, L * H)
    c_steps = np.concatenate(
        [c0, np.broadcast_to(c1[None], (steps - 1, L, batch, H)).reshape((steps - 1) * L, batch, H)], 0
    ).reshape(batch, steps, L * H)
    Wh = h_steps @ inputs["Whw"].T + inputs["Whb"]
    Wc = c_steps @ inputs["Wcw"].T + inputs["Wcb"]
    idx = np.arange(steps)[:, None] + np.arange(A)[None, :] - A
    valid = idx >= 0
    win = np.where(valid[None, :, :, None], Wh[:, np.clip(idx, 0, None)], 0.0)
    att = win + Wc[:, :, None, :]
    attn = att.mean(axis=2)
    concat_h = np.concatenate([attn, outputs], axis=2)
    outs = concat_h @ inputs["decw"].T + inputs["decb"]
    bi, ti = np.nonzero(np.arange(steps)[None, :] < (Ls[:, None] - 1))
    return outs[bi, ti].reshape(-1, V).astype(np.float32)


def _pack_inputs(inputs):
    f32 = np.float32
    emb = np.asarray(inputs["emb"], f32)
    idx0 = np.asarray(inputs["inputs"][:, 0]).astype(np.int64)
    idx1 = np.asarray(inputs["inputs"][:, 1]).astype(np.int64)

    def gates_pack(Wih, bih, bhh):
        # [65, 192]: rows 0:64 = Wih.T with (i,g,o) gate cols, row 64 = bias
        W = np.asarray(Wih, dtype=f32)
        b = np.asarray(bih, f32) + np.asarray(bhh, f32)
        out = np.zeros((65, 192), f32)
        out[0:64] = np.concatenate([W[0:H], W[2 * H:3 * H], W[3 * H:4 * H]], axis=0).T
        out[64] = np.concatenate([b[0:H], b[2 * H:3 * H], b[3 * H:4 * H]])
        return out

    p1 = np.zeros((65, _W1), f32)
    p1[0:64, _XS:_XS + 64] = emb[idx0].T
    p1[0:64, _XS + 64:_XS + 128] = emb[idx1].T
    p1[64, _XS:_XS + 128] = 1.0
    p1[:, _WIH0:_WIH0 + 192] = gates_pack(inputs["Wih0"], inputs["bih0"], inputs["bhh0"])

    p2 = np.zeros((65, _W2), f32)
    p2[:, _WIH1:_WIH1 + 192] = gates_pack(inputs["Wih1"], inputs["bih1"], inputs["bhh1"])
    Whw = np.asarray(inputs["Whw"], f32)
    Wcw = np.asarray(inputs["Wcw"], f32)
    p2[0:64, _WHW:_WHW + 64] = Whw[:, 0:H].T / A
    p2[0:64, _WHW + 64:_WHW + 128] = Whw[:, H:2 * H].T / A
    p2[64, _WHW:_WHW + 64] = np.asarray(inputs["Whb"], f32) / A
    p2[0:64, _WCW:_WCW + 64] = Wcw[:, 0:H].T
    p2[0:64, _WCW + 64:_WCW + 128] = Wcw[:, H:2 * H].T
    p2[64, _WCW:_WCW + 64] = np.asarray(inputs["Wcb"], f32)
    decw = np.asarray(inputs["decw"], f32)
    p2[0:64, _DECA:_DECA + V] = decw[:, 0:H].T       # attn rows
    p2[0:64, _DECB:_DECB + V] = decw[:, H:2 * H].T   # outputs rows
    p2[64, _DECB:_DECB + V] = np.asarray(inputs["decb"], f32)

    in_maps = []
    for core in range(NCORES):
        if core == 0:
            p20 = p2.copy()
            p20[0:64, _MVEC] = 1.0
            in_maps.append({"inp1": p1, "inp2": p20})
        else:
            in_maps.append({"inp1": p1, "inp2": p2})
    return in_maps


def kernel(**inputs):
    inputs = {k: np.asarray(v) for k, v in inputs.items()}
    Ls = np.asarray(inputs["lengths"]).astype(np.int64)
    steps = int(Ls.max())
    if steps != S or inputs["inputs"].shape != (B, S):
        return _host_reference_fallback(inputs)

    from concourse.bass_utils import run_bass_kernel_spmd

    in_maps = _pack_inputs(inputs)
    nc = _get_nc()
    res = run_bass_kernel_spmd(nc, in_maps, core_ids=list(range(NCORES)))
    outs = np.concatenate(
        [np.asarray(r["out"]).astype(np.float32).reshape(BPC, S, V)
         for r in res.results], axis=0)  # [64,512,130]

    bi, ti = np.nonzero(np.arange(steps)[None, :] < (Ls[:, None] - 1))
    return np.ascontiguousarray(outs[bi, ti].reshape(-1, V))


# revision 15
# speedup vs baseline: 1.1350x; 1.1350x over previous
"""Trainium2 Bass kernel for nn_Melody_RNN (B=64, S=512, A=20, V=130, E=H=64, L=2).

Structure exploited (all implied by the reference's exact semantics):
  * Only embedding rows for inputs[:,0] / inputs[:,1] are used; the LSTM runs
    exactly 2 timesteps (zero initial state, so the forget gate is dead).
  * The torch cat+view memory reinterpretations make h_steps/c_steps rows a
    small periodic table (period 64 in s, independent of b), with batch-0
    special rows for s<64.
  * The attention-mask bug makes softmax exactly uniform, so
    attn[b,s] = (1/A) * sum_{t=max(0,s-20)}^{s-1} Wh[b,t] + Wc[b,s].
  * outs[b,s] therefore equals generic rows OG[0:84] + 64-periodic repetition,
    with 84 special rows for batch 0 only.

Kernel v6 notes (changes vs v4 baseline):
  * TWO fused input DMAs (sync: xs+wih0 so the LSTM can start early; scalar:
    everything else). LSTM gate biases and table biases (Whb/A, Wcb) folded
    into the matmuls via a ones row (row 64 of xs / hcat / ccat).
  * Output staged in bf16 with even/odd row interleave: og2[p, 0:130] = row 2p,
    og2[p, 130:260] = row 2p+1, so each DMA descriptor moves 2 consecutive
    output rows = 520 B (stays at HBM line-rate) at half the f32 bytes.
  * 9 output DMAs (6 periodic + tail + heads + blended head0), all HWDGE
    (sync/scalar), periodic sources alternating between two partition bases
    to engage all 16 SDMA engines; host unshards and upcasts bf16 -> f32.
  * Engine queues ordered so the generic path (tables -> window tree -> decode
    -> og2 staging -> DMAs) is never stuck behind the batch-0 blend path.

SPMD: 8 cores, identical program; per-core input differs only in the mvec
column (1.0 on core 0 -> blends the batch-0 special block).
"""

import sys
import numpy as np

if "/root/.axon_site/_ro/trn_rl_repo" not in sys.path:
    sys.path.insert(0, "/root/.axon_site/_ro/trn_rl_repo")

B, S, A = 64, 512, 20
V, E, H = 130, 64, 64
NCORES = 8
BPC = B // NCORES  # batches per core

# input 1 [65, _W1]: LSTM layer-0 operands (row 64 = ones / biases)
_XS = 0        # [65, 128] = [x0T | x1T], row 64 = 1.0
_WIH0 = 128    # [65, 192] gates (i,g,o), row 64 = bih0+bhh0 packed
_W1 = 320
# input 2 [65, _W2]: everything else
_WIH1 = 0      # [65, 192] gates (i,g,o), row 64 = bih1+bhh1 packed
_WHW = 192     # [65, 128] = [Whw[:, :64].T | Whw[:, 64:].T] / A, row 64 = [Whb/A | 0]
_WCW = 320     # [65, 128] = Wcw halves transposed, row 64 = [Wcb | 0]
_DECA = 448    # [65, 130] = decw[:, 0:64].T (attn-half rows), row 64 = 0
_DECB = 578    # [65, 130] = decw[:, 64:128].T, row 64 = decb
_MVEC = 708    # [65, 1] blend selector column (1.0 on core 0)
_W2 = 709

_NC_CACHE = {}


def _build_nc():
    import concourse.bass as bass
    import concourse.bacc as bacc
    import concourse.mybir as mybir
    from concourse.tile import TileContext

    f32 = mybir.dt.float32
    bf16 = mybir.dt.bfloat16
    AF = mybir.ActivationFunctionType

    nc = bacc.Bacc("TRN2", target_bir_lowering=False, debug=False)

    d_in1 = nc.dram_tensor("inp1", [65, _W1], f32, kind="ExternalInput")
    d_in2 = nc.dram_tensor("inp2", [65, _W2], f32, kind="ExternalInput")
    d_out = nc.dram_tensor("out", [BPC * S, V], bf16, kind="ExternalOutput")

    PAD = 20 + 103  # 20 zero cols + WhSeq t=0..102
    SLOT = S * V    # elements per output slot (66560)

    with TileContext(nc) as tc:
        with (
            tc.tile_pool(name="sbuf", bufs=1) as pool,
            tc.tile_pool(name="psum", bufs=1, space="PSUM") as pp,
        ):
            # ---- input DMAs: layer-0 operands first so the LSTM can start --
            inp1 = pool.tile([65, _W1], f32)
            inp2 = pool.tile([65, _W2], f32)
            nc.sync.dma_start(out=inp1[:], in_=d_in1[:])
            nc.scalar.dma_start(out=inp2[:], in_=d_in2[:])

            # preload Sigmoid/Tanh ACT tables while the input DMAs are in flight
            ones = pool.tile([1, 2], f32)
            dummy = pool.tile([1, 2], f32)
            nc.vector.memset(ones[:], 1.0)
            nc.scalar.activation(dummy[0:1, 0:1], ones[0:1, 0:1], AF.Sigmoid)
            nc.scalar.activation(dummy[0:1, 1:2], ones[0:1, 0:1], AF.Tanh)

            # ---- persistent tiles + early memsets (off critical path) ----
            hcat = pool.tile([65, 256], f32)
            ccat = pool.tile([65, 256], f32)
            padG = pool.tile([64, PAD], f32)
            pad0 = pool.tile([64, PAD], f32)
            outG = pool.tile([65, 84], f32)
            outB = pool.tile([65, 84], f32)
            nc.vector.memset(hcat[64:65, :], 1.0)
            nc.vector.memset(ccat[64:65, :], 1.0)
            nc.gpsimd.memset(padG[:, 0:20], 0.0)
            nc.gpsimd.memset(pad0[:, 0:20], 0.0)
            nc.vector.memset(outG[64:65, :], 1.0)
            nc.vector.memset(outB[64:65, :], 1.0)

            # ---- LSTM: both timesteps batched; biases via ones-row matmul ---
            # hcat/ccat cols [l0s0|l0s1|l1s0|l1s1], row 64 = 1.0
            def lstm_layer(rhsT, wp, tag, dst_off):
                ps0 = pp.tile([128, 128], f32, tag="gates")   # [i|g]
                ps1 = pp.tile([64, 128], f32, tag="gateso")   # [o]
                nc.tensor.matmul(ps0[:], wp[:, 0:128], rhsT, start=True, stop=True)
                nc.tensor.matmul(ps1[:], wp[:, 128:192], rhsT, start=True, stop=True)
                sig_i = pool.tile([64, 128], f32, tag=f"sigi{tag}")
                tanh_g = pool.tile([64, 128], f32, tag=f"tanhg{tag}")
                sig_o = pool.tile([64, 128], f32, tag=f"sigo{tag}")
                tanh_c = pool.tile([64, 128], f32, tag=f"tanhc{tag}")
                cc = ccat[0:64, dst_off:dst_off + 128]
                hh = hcat[0:64, dst_off:dst_off + 128]
                nc.scalar.activation(tanh_g[:], ps0[64:128, :], AF.Tanh)
                nc.scalar.activation(sig_i[:], ps0[0:64, :], AF.Sigmoid)
                nc.vector.tensor_mul(cc, sig_i[:], tanh_g[:])
                nc.scalar.activation(sig_o[:], ps1[:], AF.Sigmoid)
                nc.scalar.activation(tanh_c[:], cc, AF.Tanh)
                nc.vector.tensor_mul(hh, sig_o[:], tanh_c[:])

            lstm_layer(inp1[:, _XS:_XS + 128], inp1[:, _WIH0:_WIH0 + 192], "l0", 0)
            lstm_layer(hcat[:, 0:128], inp2[:, _WIH1:_WIH1 + 192], "l1", 128)
            # hcat cols: h0l0 0:64, h1l0 64:128, h0l1 128:192, h1l1 192:256
            out0T = hcat[0:64, 128:192]
            out1T = hcat[0:64, 192:256]

            # ---- outputs-half row tiles [65, 84] (row 64 = ones for decb) --
            outZ = pool.tile([64, 84], f32)
            nc.gpsimd.tensor_copy(outG[0:64, 0:64], out1T)
            nc.gpsimd.tensor_copy(outG[0:64, 64:84], out1T[:, 0:20])
            nc.gpsimd.tensor_copy(outZ[:, 0:64], out0T)
            nc.gpsimd.tensor_copy(outZ[:, 64:84], out1T[:, 0:20])

            # ---- decoder psums (even/odd s split); outputs-half first ----
            DECB = inp2[:, _DECB:_DECB + V]          # [65, 130]
            DECA = inp2[0:64, _DECA:_DECA + V]       # [64, 130]
            ogPe = pp.tile([42, V], f32, tag="ogpe")
            ogPo = pp.tile([42, V], f32, tag="ogpo")
            obPe = pp.tile([42, V], f32, tag="obpe")
            obPo = pp.tile([42, V], f32, tag="obpo")
            # ---- all 8 row tables in 2 psums (bias via ones row) ----
            def tables(cat, wcol, tag):
                p = pp.tile([64, 128], f32, tag=tag)
                w = inp2[:, wcol:wcol + 128]
                nc.tensor.matmul(p[:], w[:, 0:64], cat[:, 0:256:2], start=True, stop=False)
                nc.tensor.matmul(p[:], w[:, 64:128], cat[:, 1:256:2], start=False, stop=True)
                return p

            whT = tables(hcat, _WHW, "tabh")
            wcT = tables(ccat, _WCW, "tabc")
            # outputs-half decode AFTER the tables so PE never interleaves
            # these into the critical tabh accumulation pair
            nc.tensor.matmul(ogPe[:], outG[:, 0:84:2], DECB, start=True, stop=False)
            nc.tensor.matmul(ogPo[:], outG[:, 1:84:2], DECB, start=True, stop=False)
            whTt = whT[:].tensor
            wcTt = wcT[:].tensor

            # ---- padded Wh sequences + Wc rows (plain strided copies) ----
            # psum col blocks: [whs0 | whp0 | whs1 | whp1]
            wcG = pool.tile([64, 84], f32)
            wc0 = pool.tile([64, 84], f32)

            def two_block(t, off):
                # [t[:, off:off+32] | t[:, off+64:off+96]] as one 3-dim AP
                return bass.AP(t, off, [[128, 64], [64, 2], [1, 32]])

            nc.vector.tensor_copy(padG[:, 20:84], two_block(whTt, 32))
            nc.vector.tensor_copy(padG[:, 84:103], bass.AP(whTt, 32, [[128, 64], [1, 19]]))
            nc.scalar.copy(pad0[:, 20:84], two_block(whTt, 0))
            nc.scalar.copy(pad0[:, 84:103], bass.AP(whTt, 32, [[128, 64], [1, 19]]))
            nc.scalar.copy(wcG[:, 0:64], two_block(wcTt, 32))
            nc.scalar.copy(wcG[:, 64:84], bass.AP(wcTt, 32, [[128, 64], [1, 20]]))
            nc.scalar.copy(wc0[:, 0:64], two_block(wcTt, 0))
            nc.scalar.copy(wc0[:, 64:84], bass.AP(wcTt, 32, [[128, 64], [1, 20]]))

            # ---- sliding 20-window sums via shift-add tree ----
            def window20(pad, eng, tag):
                t1 = pool.tile([64, 102], f32, tag=f"t1{tag}")
                t2 = pool.tile([64, 100], f32, tag=f"t2{tag}")
                t4 = pool.tile([64, 96], f32, tag=f"t4{tag}")
                t8 = pool.tile([64, 88], f32, tag=f"t8{tag}")
                w20 = pool.tile([64, 84], f32, tag=f"w20{tag}")
                eng.tensor_add(t1[:], pad[:, 0:102], pad[:, 1:103])
                eng.tensor_add(t2[:], t1[:, 0:100], t1[:, 2:102])
                eng.tensor_add(t4[:], t2[:, 0:96], t2[:, 4:100])
                eng.tensor_add(t8[:], t4[:, 0:88], t4[:, 8:96])
                eng.tensor_add(w20[:], t8[:, 0:84], t2[:, 16:100])
                return w20

            w20G = window20(padG, nc.vector, "g")
            w20_0 = window20(pad0, nc.gpsimd, "z")

            # ---- attn halves [64, 84] ----
            attnG = pool.tile([64, 84], f32)
            attnZ = pool.tile([64, 84], f32)
            nc.vector.tensor_add(attnG[:], w20G[:], wcG[:])
            nc.gpsimd.tensor_add(attnZ[:], w20_0[:], wc0[:])

            # ---- finish generic decode ----
            nc.tensor.matmul(ogPe[:], attnG[:, 0:84:2], DECA, start=False, stop=True)
            nc.tensor.matmul(ogPo[:], attnG[:, 1:84:2], DECA, start=False, stop=True)

            # ---- bf16 staging: og2[p, 0:130] = row 2p, [130:260] = row 2p+1.
            #      Partitions 64:106 = aligned compute replica so half the
            #      periodic DMAs read odd-numbered SDMA engines' partitions
            #      (engine k serves fixed SBUF partitions; 64:128 -> odd). ----
            og2 = pool.tile([106, 260], bf16)
            ob2 = pool.tile([42, 260], bf16)
            nc.scalar.copy(og2[0:42, 0:130], ogPe[:])
            nc.scalar.copy(og2[0:42, 130:260], ogPo[:])
            nc.vector.tensor_copy(og2[64:106, :], og2[0:42, :])
            og2t = og2[:].tensor
            ob2t = ob2[:].tensor

            # ---- output DMAs (bf16, 520B descriptors = 2 rows each),
            #      all on HWDGE queues (SWDGE drains far slower) ----
            def dst(row0, nparts, slot0, nslots):
                return bass.AP(d_out, slot0 * SLOT + row0 * V,
                               [[260, nparts], [SLOT, nslots], [1, 260]])

            def src(t, part0, nparts, nslots):
                return bass.AP(t, part0 * 260, [[260, nparts], [0, nslots], [1, 260]])

            # periodic rows 84+64k <- OG[20:84] (k = 0..5), all 8 slots each;
            # even k from the base copy (parts 10:42), odd k from the replica
            # (parts 74:106) to spread across all 16 SDMA engines
            nc.sync.dma_start(out=dst(84, 32, 0, 8), in_=src(og2t, 10, 32, 8))
            nc.scalar.dma_start(out=dst(148, 32, 0, 8), in_=src(og2t, 74, 32, 8))
            nc.sync.dma_start(out=dst(212, 32, 0, 8), in_=src(og2t, 10, 32, 8))
            nc.scalar.dma_start(out=dst(276, 32, 0, 8), in_=src(og2t, 74, 32, 8))
            nc.sync.dma_start(out=dst(340, 32, 0, 8), in_=src(og2t, 10, 32, 8))
            nc.scalar.dma_start(out=dst(404, 32, 0, 8), in_=src(og2t, 74, 32, 8))
            # tail rows 468:512 <- OG[20:64] (from the replica: odd engines)
            nc.sync.dma_start(out=dst(468, 22, 0, 8), in_=src(og2t, 74, 22, 8))
            # heads: slots 1..7 generic
            nc.sync.dma_start(out=dst(0, 42, 1, 7), in_=src(og2t, 0, 42, 7))

            # ---- blend batch-0 variants: X_B = X_G + mvec*(X_0 - X_G) ----
            MV = inp2[0:64, _MVEC:_MVEC + 1]
            attnB = pool.tile([64, 84], f32)
            dA = pool.tile([64, 84], f32)
            dO = pool.tile([64, 84], f32)
            nc.vector.tensor_sub(dO[:], outZ[:], outG[0:64, :])
            nc.vector.tensor_scalar_mul(dO[:], dO[:], MV)
            nc.vector.tensor_add(outB[0:64, :], outG[0:64, :], dO[:])
            nc.vector.tensor_sub(dA[:], attnZ[:], attnG[:])
            nc.vector.tensor_scalar_mul(dA[:], dA[:], MV)
            nc.vector.tensor_add(attnB[:], attnG[:], dA[:])

            nc.tensor.matmul(obPe[:], outB[:, 0:84:2], DECB, start=True, stop=False)
            nc.tensor.matmul(obPo[:], outB[:, 1:84:2], DECB, start=True, stop=False)
            nc.tensor.matmul(obPe[:], attnB[:, 0:84:2], DECA, start=False, stop=True)
            nc.tensor.matmul(obPo[:], attnB[:, 1:84:2], DECA, start=False, stop=True)
            nc.vector.tensor_copy(ob2[:, 0:130], obPe[:])
            nc.vector.tensor_copy(ob2[:, 130:260], obPo[:])

            # head slot 0 blended
            nc.scalar.dma_start(
                out=bass.AP(d_out, 0, [[260, 42], [1, 260]]),
                in_=bass.AP(ob2t, 0, [[260, 42], [1, 260]]))

    nc.compile()
    return nc


def _get_nc():
    if "nc" not in _NC_CACHE:
        _NC_CACHE["nc"] = _build_nc()
    return _NC_CACHE["nc"]


def _host_reference_fallback(inputs):
    """Pure-numpy replica of the reference for steps != 512 (never hit with the
    canonical setup_inputs, which fixes lengths = 512)."""
    emb = inputs["emb"]; L = 2
    Ls = np.asarray(inputs["lengths"]); steps = int(Ls.max()); batch = inputs["inputs"].shape[0]
    layers = [(inputs["Wih0"], inputs["bih0"], inputs["bhh0"]),
              (inputs["Wih1"], inputs["bih1"], inputs["bhh1"])]
    sig = lambda z: 1.0 / (1.0 + np.exp(-z))

    def step(x):
        hs, cs = [], []
        inp = x
        for (Wih, bih, bhh) in layers:
            g = inp @ Wih.T + bih + bhh
            i, f, gg, o = np.split(g, 4, axis=-1)
            c = sig(i) * np.tanh(gg)
            h = sig(o) * np.tanh(c)
            hs.append(h); cs.append(c); inp = h
        return inp.astype(np.float32), np.stack(hs).astype(np.float32), np.stack(cs).astype(np.float32)

    x0 = emb[inputs["inputs"][:, 0]]
    x1 = emb[inputs["inputs"][:, 1]]
    out0, h0, c0 = step(x0)
    out1, h1, c1 = step(x1)
    outputs = np.concatenate(
        [out0[None], np.broadcast_to(out1[None], (steps - 1, batch, H))], 0
    ).reshape(batch, steps, H)
    h_steps = np.concatenate(
        [h0, np.broadcast_to(h1[None], (steps - 1, L, batch, H)).reshape((steps - 1) * L, batch, H)], 0
    ).reshape(batch, steps, L * H)
    c_steps = np.concatenate(
        [c0, np.broadcast_to(c1[None], (steps - 1, L, batch, H)).reshape((steps - 1) * L, batch, H)], 0
    ).reshape(batch, steps, L * H)
    Wh = h_steps @ inputs["Whw"].T + inputs["Whb"]
    Wc = c_steps @ inputs["Wcw"].T + inputs["Wcb"]
    idx = np.arange(steps)[:, None] + np.arange(A)[None, :] - A
    valid = idx >= 0
    win = np.where(valid[None, :, :, None], Wh[:, np.clip(idx, 0, None)], 0.0)
    att = win + Wc[:, :, None, :]
    attn = att.mean(axis=2)
    concat_h = np.concatenate([attn, outputs], axis=2)
    outs = concat_h @ inputs["decw"].T + inputs["decb"]
    bi, ti = np.nonzero(np.arange(steps)[None, :] < (Ls[:, None] - 1))
    return outs[bi, ti].reshape(-1, V).astype(np.float32)


def _pack_inputs(inputs):
    f32 = np.float32
    emb = np.asarray(inputs["emb"], f32)
    idx0 = np.asarray(inputs["inputs"][:, 0]).astype(np.int64)
    idx1 = np.asarray(inputs["inputs"][:, 1]).astype(np.int64)

    def gates_pack(Wih, bih, bhh):
        # [65, 192]: rows 0:64 = Wih.T with (i,g,o) gate cols, row 64 = bias
        W = np.asarray(Wih, dtype=f32)
        b = np.asarray(bih, f32) + np.asarray(bhh, f32)
        out = np.zeros((65, 192), f32)
        out[0:64] = np.concatenate([W[0:H], W[2 * H:3 * H], W[3 * H:4 * H]], axis=0).T
        out[64] = np.concatenate([b[0:H], b[2 * H:3 * H], b[3 * H:4 * H]])
        return out

    p1 = np.zeros((65, _W1), f32)
    p1[0:64, _XS:_XS + 64] = emb[idx0].T
    p1[0:64, _XS + 64:_XS + 128] = emb[idx1].T
    p1[64, _XS:_XS + 128] = 1.0
    p1[:, _WIH0:_WIH0 + 192] = gates_pack(inputs["Wih0"], inputs["bih0"], inputs["bhh0"])

    p2 = np.zeros((65, _W2), f32)
    p2[:, _WIH1:_WIH1 + 192] = gates_pack(inputs["Wih1"], inputs["bih1"], inputs["bhh1"])
    Whw = np.asarray(inputs["Whw"], f32)
    Wcw = np.asarray(inputs["Wcw"], f32)
    p2[0:64, _WHW:_WHW + 64] = Whw[:, 0:H].T / A
    p2[0:64, _WHW + 64:_WHW + 128] = Whw[:, H:2 * H].T / A
    p2[64, _WHW:_WHW + 64] = np.asarray(inputs["Whb"], f32) / A
    p2[0:64, _WCW:_WCW + 64] = Wcw[:, 0:H].T
    p2[0:64, _WCW + 64:_WCW + 128] = Wcw[:, H:2 * H].T
    p2[64, _WCW:_WCW + 64] = np.asarray(inputs["Wcb"], f32)
    decw = np.asarray(inputs["decw"], f32)
    p2[0:64, _DECA:_DECA + V] = decw[:, 0:H].T       # attn rows
    p2[0:64, _DECB:_DECB + V] = decw[:, H:2 * H].T   # outputs rows
    p2[64, _DECB:_DECB + V] = np.asarray(inputs["decb"], f32)

    in_maps = []
    for core in range(NCORES):
        if core == 0:
            p20 = p2.copy()
            p20[0:64, _MVEC] = 1.0
            in_maps.append({"inp1": p1, "inp2": p20})
        else:
            in_maps.append({"inp1": p1, "inp2": p2})
    return in_maps


def kernel(**inputs):
    inputs = {k: np.asarray(v) for k, v in inputs.items()}
    Ls = np.asarray(inputs["lengths"]).astype(np.int64)
    steps = int(Ls.max())
    if steps != S or inputs["inputs"].shape != (B, S):
        return _host_reference_fallback(inputs)

    from concourse.bass_utils import run_bass_kernel_spmd

    in_maps = _pack_inputs(inputs)
    nc = _get_nc()
    res = run_bass_kernel_spmd(nc, in_maps, core_ids=list(range(NCORES)))
    outs = np.concatenate(
        [np.asarray(r["out"]).astype(np.float32).reshape(BPC, S, V)
         for r in res.results], axis=0)  # [64,512,130]

    bi, ti = np.nonzero(np.arange(steps)[None, :] < (Ls[:, None] - 1))
    return np.ascontiguousarray(outs[bi, ti].reshape(-1, V))
